# revision 1
# baseline (speedup 1.0000x reference)
"""KAN GLU expert (nn_KAN_GLUExpert) — TRN2 Bass kernel, 8 NeuronCores.

Math: reference kan_linear(x, bw, sw, grid) = silu(x) @ bw.T + einsum('nic,oic->no', b_splines(x), sw)
with a uniform shared grid (h=0.4 on [-2.2, 2.2], cubic, 8 basis fns). On a uniform grid the
8 spline bases are shifted copies of the cardinal cubic B-spline:
    B_c(x) = r^3/6 - (2/3) q^3,  r = relu(2 - t), q = relu(1 - t),  t = |2.5 x + 3.5 - c|
(verified to fp32 rounding against the Cox-de Boor recursion, incl. exact knots / out-of-domain).

base + spline fuse into ONE matmul over K = 9*in: slab 0 = silu(x), slabs 1..8 = B_c(x).
Layers 1+2: spline slabs and weights in fp8e4m3 (weights prescaled x256; PSUM is 256x and
the GLU multiply folds in 2^-16), base slab+weights bf16, fp8 matmuls use DoubleRow perf
mode (2 k-tiles per instruction, ~1.4x PE throughput). Layer 3: bf16 spline slabs/weights +
f32r base (layer-3 quantization dominates end-to-end error, so it stays 16-bit).

Sharding: data-parallel over tokens. Each of the 8 cores takes 512 of the 4096 tokens and
streams all weights once; no collective. Output slices are concatenated on host.
"""
import numpy as np
import ml_dtypes

import concourse.bacc as bacc
import concourse.mybir as mybir
import concourse.tile as tile
from concourse.bass_utils import run_bass_kernel_spmd

F32 = mybir.dt.float32
F32R = mybir.dt.float32r
BF16 = mybir.dt.bfloat16
F8 = mybir.dt.float8e4
AF = mybir.ActivationFunctionType
ALU = mybir.AluOpType
DR = mybir.MatmulPerfMode.DoubleRow

DM = 1024          # d_model
DF = 4096          # d_ff
C = 8              # spline coefficients per input
NCORES = 8
TOK = 512          # tokens per core
NPAIR = 16         # L12 row-pairs: 256 rows of w1 + 256 rows of w2 each
KI1 = DM // 128    # 8 k-tiles per slab, layer 1
KI3 = DF // 128    # 32 k-tiles per slab, layer 3

WSCALE = 256.0     # L12 weight prescale so fp8 spline weights stay in e4m3 normal range
GLU_S = 1.0 / (WSCALE * WSCALE)
USE_POOL = False   # run the two basis cubes on gpsimd (Pool) instead of DVE
L3F8 = 8           # number of layer-3 spline slabs (c < L3F8) in fp8-DoubleRow
L3_S = 256.0 if L3F8 else 1.0   # layer-3 weight prescale (power of 2, exact in bf16)

_BF16 = ml_dtypes.bfloat16
_F8 = ml_dtypes.float8_e4m3   # TRN fp8_exp4 semantics (max normal 240)


def _register_const(nc, value, dtype=F32):
    key = (dtype, float(value))
    if key in nc.const_aps.aps:
        return
    t = nc.alloc_sbuf_tensor(f"const-{dtype.name}-{value}", [128, 1], dtype)
    nc.gpsimd.memset(t.ap(), float(value))
    nc.const_aps.aps[key] = t.ap()


SQRT6I = float(6.0 ** -0.5)          # (2-t)/sqrt(6) squared -> (2-t)^2/6
SQ23 = float((2.0 / 3.0) ** 0.5)     # sqrt(2/3)(1-t) squared -> (2/3)(1-t)^2


def _basis_ops(nc, wst, wsb, out_ap, x_ap, c, shape):
    """Emit ops computing B_c slab for x_ap into out_ap (bf16 or fp8).
    Balanced 4 Act + 4 DVE ops: Square is computed UNclamped ((2-t)^2/6 for all
    t) — multiplying by the clamped Relu factor zeroes everything past the
    support, so the product r^2/6 * r = r^3/6 is exact. t stays fp32 (|t| can
    reach ~11); the rest is bf16."""
    t = wst.tile(shape, F32, tag="wst", name=f"t_{c}")
    nc.scalar.activation(t[:], x_ap, AF.Abs, bias=float(3.5 - c), scale=2.5)
    r = wsb.tile(shape, BF16, tag="wsb", name=f"r_{c}")
    nc.scalar.activation(r[:], t[:], AF.Relu, bias=2.0, scale=-1.0)         # r = relu(2-t)
    # r2 stays on Act (Square): A/B'd three times — moving it to a DVE
    # scalar_tensor_tensor (2 read ports, no bf16 double-pump) reads worse.
    r2 = wsb.tile(shape, BF16, tag="wsb", name=f"r2_{c}")
    nc.scalar.activation(r2[:], t[:], AF.Square, bias=2.0 * SQRT6I, scale=-SQRT6I)  # (2-t)^2/6
    q = wsb.tile(shape, BF16, tag="wsb", name=f"q_{c}")
    nc.vector.tensor_scalar(q[:], r[:], 1.0, 0.0, ALU.subtract, ALU.max)    # q = relu(r-1) = relu(1-t)
    q2 = wsb.tile(shape, BF16, tag="wsb", name=f"q2_{c}")
    nc.vector.scalar_tensor_tensor(q2[:], q[:], 2.0 / 3.0, q[:], ALU.mult, ALU.mult)  # (2/3) q^2
    u = wsb.tile(shape, BF16, tag="wsb", name=f"u_{c}")
    nc.vector.tensor_mul(u[:], r2[:], r[:])                                 # r^3/6
    v = wsb.tile(shape, BF16, tag="wsb", name=f"v_{c}")
    nc.vector.tensor_mul(v[:], q2[:], q[:])                                 # (2/3) q^3
    nc.vector.tensor_sub(out_ap, u[:], v[:])                                # B = r^3/6 - (2/3) q^3


def build_program(repeat=1):
    nc = bacc.Bacc("TRN2", target_bir_lowering=False, debug=False, num_devices=NCORES)

    xs_d = nc.dram_tensor("xs", (128, KI1, TOK), F32, kind="ExternalInput")
    wb12_d = nc.dram_tensor("wb12", (NPAIR, 128, KI1, 512), BF16, kind="ExternalInput")
    ws12_d = nc.dram_tensor("ws12", (NPAIR, C, 128, KI1, 512), F8, kind="ExternalInput")
    wb3_d = nc.dram_tensor("wb3", (16, 128, 2, 1024), BF16, kind="ExternalInput")
    ws3_d = (nc.dram_tensor("ws3", (C - L3F8, 8, 128, 4, 1024), BF16, kind="ExternalInput")
             if L3F8 < C else None)
    ws3f8_d = (nc.dram_tensor("ws3f8", (L3F8, 8, 128, 4, 1024), F8, kind="ExternalInput")
               if L3F8 else None)
    out_d = nc.dram_tensor("out", (128, 8, TOK), F32, kind="ExternalOutput")

    for c in range(C):
        _register_const(nc, 3.5 - c)
    for v in (2.0, 1.0, 2.0 * SQRT6I):
        _register_const(nc, v)
    nc.all_engine_barrier()

    with tile.TileContext(nc) as tc:
      for _rep in range(repeat):   # >1 only for exec-time measurement
        with tc.tile_pool(name="hpool", bufs=1) as hpool:
            h = hpool.tile([128, KI3, TOK], BF16, name="h")

            # ---------------- layers 1+2 (GLU halves) ----------------
            with (
                tc.tile_pool(name="slabs1", bufs=1) as slabs1,
                tc.tile_pool(name="wload_b", bufs=2) as wload_b,
                tc.tile_pool(name="wload_s", bufs=2) as wload_s,
                tc.tile_pool(name="ps12", bufs=8, space="PSUM") as ps12,
                tc.tile_pool(name="glu_tmp", bufs=2) as glu_tmp,
            ):
                silu1 = slabs1.tile([128, KI1, TOK], BF16, name="silu1")
                B1 = [slabs1.tile([128, KI1, TOK], F8, tag=f"B1_{c}", name=f"B1_{c}")
                      for c in range(C)]

                wb_t, ws_t = {}, {}

                def issue_wb(j):
                    wb_t[j] = wload_b.tile([128, KI1, 512], BF16, tag="wb", name=f"wb_{j}")
                    nc.sync.dma_start(wb_t[j][:], wb12_d[j])

                def issue_ws(j, pool=None):
                    ws_t[j] = []
                    for c in range(C):
                        t = (pool or wload_s).tile([128, KI1, 512], F8,
                                                   tag=f"ws{c}", name=f"ws_{j}_{c}")
                        nc.sync.dma_start(t[:], ws12_d[j, c])
                        ws_t[j].append(t)

                def issue_w12(j):
                    issue_wb(j)
                    issue_ws(j)

                def base_mms(j):
                    wbj = wb_t[j]
                    for ki in range(KI1):
                        for m in range(4):
                            nc.tensor.matmul(
                                acc_t[j][m][:], wbj[:, ki, 128 * m:128 * (m + 1)],
                                silu1[:, ki, :], start=(ki == 0), stop=False)

                def spline_glu(j, qmajor=False):
                    # spline: fp8 B slabs x fp8 spline weights, DoubleRow.
                    # Strip-major (q outer) for j=0/1 so they stream behind basis
                    # production; c-major for the rest so each ws_{j,c} tile frees
                    # as soon as its 16 matmuls retire (finer DMA pipelining).
                    wsj, acc = ws_t.pop(j), acc_t.pop(j)
                    wb_t.pop(j)
                    order = ([(q, c) for q in range(KI1 // 2) for c in range(C)]
                             if qmajor else
                             [(q, c) for c in range(C) for q in range(KI1 // 2)])
                    for n, (q, c) in enumerate(order):
                        last = (n == len(order) - 1)
                        for m in range(4):
                            nc.tensor.matmul(
                                acc[m][:],
                                wsj[c][:, 2 * q:2 * q + 2, 128 * m:128 * (m + 1)],
                                B1[c][:, 2 * q:2 * q + 2, :],
                                start=False, stop=last, perf_mode=DR)
                    # GLU: h rows [256j, 256j+256) = L1 * L2 (undo the 256^2 weight scale)
                    for t in range(2):
                        tmp = glu_tmp.tile([128, TOK], F32, tag="gt", name=f"gt_{j}_{t}")
                        nc.scalar.copy(tmp[:], acc[t][:])
                        nc.vector.scalar_tensor_tensor(
                            h[:, 2 * j + t, :], tmp[:], GLU_S, acc[2 + t][:],
                            ALU.mult, ALU.mult)

                acc_t = {}

                def alloc_acc(j):
                    acc_t[j] = [ps12.tile([128, TOK], F32, tag="ps", name=f"ps_{j}_{m}")
                                for m in range(4)]

                # x + silu strips first, then j0/j1 base matmuls so PE has work
                # while the basis chain fills B1; basis production is strip-major
                # and j0/j1's DR loops consume strip-major right behind it.
                with (
                    tc.tile_pool(name="xload", bufs=1) as xload,
                    tc.tile_pool(name="ws1t", bufs=2) as ws1t,
                    tc.tile_pool(name="ws1b", bufs=12) as ws1b,
                ):
                    xs = xload.tile([128, KI1, TOK], F32, name="xs_t")
                    for s in range(KI1):
                        nc.sync.dma_start(xs[:, s:s + 1, :], xs_d[:, s:s + 1, :])
                        if s == 1:
                            issue_wb(0)   # first base matmul needs only x0+wb0
                    issue_wb(1)
                    issue_ws(0)
                    issue_ws(1)
                    for s in range(KI1):
                        nc.scalar.activation(silu1[:, s:s + 1, :],
                                             xs[:, s:s + 1, :], AF.Silu)
                    alloc_acc(0)
                    base_mms(0)
                    alloc_acc(1)
                    base_mms(1)
                    # basis in strip-PAIRS: matches the DR consumption granularity
                    # (q needs exactly strips 2q,2q+1) and halves per-op overhead
                    for p in range(KI1 // 2):
                        for c in range(C):
                            _basis_ops(nc, ws1t, ws1b, B1[c][:, 2 * p:2 * p + 2, :],
                                       xs[:, 2 * p:2 * p + 2, :], c, [128, 2, TOK])

                # j2's spline weights go in a third buffer set carved out of the
                # just-freed basis workspace, so their DMAs start immediately
                # instead of waiting for j0's buffers (which q-major holds until
                # the whole basis phase completes).
                with tc.tile_pool(name="wload_s2", bufs=1) as wload_s2:
                    issue_ws(2, pool=wload_s2)
                    for j in range(NPAIR):
                        if j >= 2:
                            alloc_acc(j)
                            base_mms(j)
                        if j == 0:
                            issue_wb(2)
                        elif j + 2 < NPAIR:
                            issue_w12(j + 2)
                        spline_glu(j, qmajor=(j < 2))

            # ---------------- layer 3 ----------------
            with (
                tc.tile_pool(name="sil3", bufs=3) as sil3p,
                tc.tile_pool(name="b3", bufs=6) as b3p,
                tc.tile_pool(name="ws3t", bufs=2) as ws3t,
                tc.tile_pool(name="ws3b", bufs=12) as ws3b,
                tc.tile_pool(name="w3load_b", bufs=2) as w3load_b,
                tc.tile_pool(name="w3load_s", bufs=4) as w3load_s,
                tc.tile_pool(name="ps3", bufs=1, space="PSUM") as ps3,
                tc.tile_pool(name="outp", bufs=1) as outp,
            ):
                acc3 = [ps3.tile([128, TOK], F32, tag=f"o{m}", name=f"ps3_{m}")
                        for m in range(8)]
                # base part: silu(h) strips of 4 k-tiles
                for s in range(KI3 // 4):
                    sil = sil3p.tile([128, 4, TOK], BF16, tag="sil", name=f"sil_{s}")
                    nc.scalar.activation(sil[:], h[:, 4 * s:4 * s + 4, :], AF.Silu)
                    for half in range(2):
                        wt = w3load_b.tile([128, 2, 1024], BF16, tag="w3b", name=f"w3b_{s}_{half}")
                        nc.sync.dma_start(wt[:], wb3_d[2 * s + half])
                        for r in range(2):
                            ki = 4 * s + 2 * half + r
                            for m in range(8):
                                nc.tensor.matmul(
                                    acc3[m][:], wt[:, r, 128 * m:128 * (m + 1)],
                                    sil[:, 2 * half + r, :], start=(ki == 0), stop=False)
                # spline part (basis per full 4-k-tile strip); slabs c < L3F8 run
                # fp8-DoubleRow (weights prescaled x256), the rest bf16
                for c in range(C):
                    fp8 = c < L3F8
                    for s in range(KI3 // 4):
                        bt = b3p.tile([128, 4, TOK], F8 if fp8 else BF16,
                                      tag="b3", name=f"b3_{c}_{s}")
                        _basis_ops(nc, ws3t, ws3b, bt[:],
                                   h[:, 4 * s:4 * s + 4, :], c, [128, 4, TOK])
                        wt = w3load_s.tile([128, 4, 1024], F8 if fp8 else BF16,
                                           tag="w3s", name=f"w3s_{c}_{s}")
                        nc.sync.dma_start(wt[:], ws3f8_d[c, s] if fp8
                                          else ws3_d[c - L3F8, s])
                        last_cs = (c == C - 1 and s == KI3 // 4 - 1)
                        if fp8:
                            for u2 in range(2):
                                for m in range(8):
                                    nc.tensor.matmul(
                                        acc3[m][:],
                                        wt[:, 2 * u2:2 * u2 + 2, 128 * m:128 * (m + 1)],
                                        bt[:, 2 * u2:2 * u2 + 2, :], start=False,
                                        stop=(last_cs and u2 == 1), perf_mode=DR)
                        else:
                            for r in range(4):
                                for m in range(8):
                                    nc.tensor.matmul(
                                        acc3[m][:], wt[:, r, 128 * m:128 * (m + 1)],
                                        bt[:, r, :], start=False,
                                        stop=(last_cs and r == 3))
                # copy out, undoing the x256 weight prescale (4 chunked DMAs so
                # the tail drains in parallel)
                ostage = outp.tile([128, 8, TOK], F32, name="ostage")
                for m in range(8):
                    nc.scalar.activation(ostage[:, m, :], acc3[m][:], AF.Copy,
                                         scale=1.0 / L3_S)
                    if m % 2 == 1:
                        nc.sync.dma_start(out_d[:, m - 1:m + 1, :],
                                          ostage[:, m - 1:m + 1, :])

    nc.compile()
    return nc


def pack_weights(base_w1, spline_w1, base_w2, spline_w2, base_w3, spline_w3):
    f32 = np.float32
    # WB12: (16, 128, 8, 512) bf16, x256 — cols = [w1 rows 256j.., w2 rows 256j..]
    w12 = np.concatenate([np.asarray(base_w1, f32).reshape(NPAIR, 256, DM),
                          np.asarray(base_w2, f32).reshape(NPAIR, 256, DM)], axis=1)  # (16, 512, 1024) [j, m, k]
    wb12 = np.ascontiguousarray(
        (w12 * WSCALE).reshape(NPAIR, 512, KI1, 128).transpose(0, 3, 2, 1)).astype(_BF16)

    # WS12: (16, 8, 128, 8, 512) fp8e4m3, x256
    s12 = np.concatenate([np.asarray(spline_w1, f32).reshape(NPAIR, 256, DM, C),
                          np.asarray(spline_w2, f32).reshape(NPAIR, 256, DM, C)], axis=1)  # (16, 512, 1024, 8)
    ws12 = np.ascontiguousarray(
        np.clip(s12 * WSCALE, -240, 240)
        .reshape(NPAIR, 512, KI1, 128, C).transpose(0, 4, 3, 2, 1)).astype(_F8)  # (16, C, 128, 8, 512)

    # WB3: (16, 128, 2, 1024) bf16, x L3_S: base_w3 (1024, 4096): [m, k]
    wb3 = np.ascontiguousarray(
        (np.asarray(base_w3, f32) * L3_S).T
        .reshape(16, 2, 128, DM).transpose(0, 2, 1, 3)).astype(_BF16)  # (16, 128, 2, 1024)

    # WS3: (8, 8, 128, 4, 1024), x L3_S: spline_w3 (1024, 4096, 8);
    # slabs c < L3F8 ship as fp8e4m3, the rest bf16
    ws3all = np.ascontiguousarray(
        np.clip(np.asarray(spline_w3, f32).transpose(2, 1, 0) * L3_S, -240, 240)
        .reshape(C, 8, 4, 128, DM).transpose(0, 1, 3, 2, 4))  # (8, 8, 128, 4, 1024)
    packed = {"wb12": wb12, "ws12": ws12, "wb3": wb3}
    if L3F8:
        packed["ws3f8"] = ws3all[:L3F8].astype(_F8)
    if L3F8 < C:
        packed["ws3"] = ws3all[L3F8:].astype(_BF16)
    return packed


def make_in_maps(x2, packed):
    in_maps = []
    for cidx in range(NCORES):
        xs = np.ascontiguousarray(
            x2[cidx * TOK:(cidx + 1) * TOK].T.reshape(KI1, 128, TOK).transpose(1, 0, 2))
        m = {"xs": xs}
        m.update(packed)
        in_maps.append(m)
    return in_maps


_prog_cache = {}


def kernel(x, base_w1, spline_w1, base_w2, spline_w2, base_w3, spline_w3,
           grid_in=None, grid_ff=None):
    x = np.asarray(x, np.float32)
    shp = x.shape
    x2 = x.reshape(-1, DM)                       # (4096, 1024)
    ntok = x2.shape[0]
    assert ntok == NCORES * TOK

    packed = pack_weights(base_w1, spline_w1, base_w2,
                          spline_w2, base_w3, spline_w3)

    if "nc" not in _prog_cache:
        _prog_cache["nc"] = build_program()
    nc = _prog_cache["nc"]

    in_maps = make_in_maps(x2, packed)

    res = run_bass_kernel_spmd(nc, in_maps, core_ids=list(range(NCORES)))

    out = np.empty((ntok, DM), np.float32)
    for cidx in range(NCORES):
        o = res.results[cidx]["out"]             # (128, 8, 512)
        out[cidx * TOK:(cidx + 1) * TOK] = o.transpose(1, 0, 2).reshape(DM, TOK).T
    return out.reshape(shp)

